# revision 43
# baseline (speedup 1.0000x reference)
"""Dispersion loss kernel for 8x TRN2 NeuronCores (Bass/Tile), v2.

Math: rows of class_centroid [8192, 2048] are L2-normalized; the loss is
  mean_i( sum_j exp(-||xn_i - xn_j||^2) / (N-1) )
    = (1/(N*(N-1))) * sum_{i,j} exp(2*cos_ij - 2)

v2 reformulation (validated at 1.05e-5 rel err vs the f32 reference):
work with the RAW fp8-quantized rows x8 = fp8(x) and post-normalize,
  cos_ij = (x8_i . x8_j) * rinv_i * rinv_j,   rinv = 1/||x8||.
The host pre-casts to fp8 e4m3 and pre-transposes each 512-row block into
the K-major matmul layout [128, 16, 512], so the device does NO bulk
normalize/cast/transpose work for the stationary side at all:

- ssq (needed for rinv of all 10 cached blocks) comes from the PE: per
  block an extra fp8 DoubleRow matmul chain accumulates the 4 [128,128]
  Gram-diagonal subtiles into one PSUM bank; a DVE scalar_tensor_tensor
  against a [128,128] identity mask with fused accum extracts
  diag -> ssq (f32). rinv' = exp(-0.5*ln(ssq)+b) on ScalarE.
- pair roles are flipped vs v1: stationary = sj (raw, all 10 slots),
  moving = si in {0,1}, so pairs (0,j),(1,j) share one LDWEIGHTS
  (deduped post-schedule). The moving-side 1/norm: 16*rinv rows are
  replicated across partitions via PE transpose + K=1 ones-matmul
  (rrepb); early sweeps (sj < XTN_FROM) run RAW moving tiles and fold
  the norm into a per-tile DVE mult (g2 = G * rrepb) before the exp,
  later sweeps consume pre-normalized fp8 moving tiles (xtn) whose DVE
  production is spread across the early iterations. The stationary-side
  1/norm rides the epilogue activation as a per-partition vector scale:
  exp(G'*(rinv_m/8) - 2), with fused row-sum accumulation into partials.

Per-core: 10 fp8 DMA loads, 40 diag-chain + 576 pair fp8 DoubleRow
matmuls (K=256/instr, 72 PSUM tiles), 40 DVE diag extracts, 44 g2 mults
+ 16 xtn chunk mults, 72 ACT Exp epilogues. Host reduces the 8 partial
tensors in f64. build_program(probe=...) builds gutted timing probes.

The walrus build in this container predates this bass: _sem_clear_compat
and _split_multi_waits patch around unsupported opcodes.
"""

import numpy as np

import concourse.bass as bass
import concourse.mybir as mybir
from concourse.tile import TileContext
from concourse.bass_utils import run_bass_kernel_spmd

F32 = mybir.dt.float32
BF16 = mybir.dt.bfloat16
FP8 = mybir.dt.float8e4
FP8_SCALE = 16.0


# --------------------------------------------------------------------------
# Compatibility shims for the walrus compiler build in this container:
# 1) EVENT_SEMAPHORE_RANGE_CLEAR (opcode 176) is not understood -> emit
#    per-semaphore EventSemaphore sem-wr-imm 0 instead.
# 2) Instructions with >1 sync waits ("Too many sync wait commands") ->
#    split extra waits onto single-wait EventSemaphore carriers.
# --------------------------------------------------------------------------
def _sem_clear_compat(self, sem):
    nums = (
        list(sem) if isinstance(sem, range)
        else [sem.num if hasattr(sem, "num") else int(sem)]
    )
    inst = None
    for n in nums:
        inst = mybir.InstEventSemaphore(
            name=f"semclr_{self.bass.next_id()}",
            engine=self.engine,
            ins=[],
            outs=[],
            sync_info=mybir.SyncInfo(
                on_wait=[],
                on_update=[
                    mybir.SyncUpdate(
                        sync_type="semaphore",
                        id=n,
                        ant_name=f"semclr{n}",
                        update_mode="sem-wr-imm",
                        update_value=0,
                    )
                ],
            ),
            bass_nofuse=True,
        )
        self.add_instruction(inst)
    return inst


bass.BassGpSimd.sem_clear = _sem_clear_compat


def _dedup_ldweights(nc):
    """Remove consecutive PE LDWEIGHTS with identical source APs: the weights
    are already resident in the array, so repeated loads between matmuls that
    share a stationary tile are pure overhead. Non-empty sync_info on removed
    loads is preserved on a zero-cost EventSemaphore carrier."""
    def sig(i):
        ap = i.ins[0]
        return (
            getattr(ap, "memref", None), getattr(ap, "offset", None),
            str(getattr(ap, "ap", None)), str(getattr(ap, "dtype", None)),
            i.tile_position, i.perf_mode, i.is_transpose,
        )
    removed = 0
    for bb in nc.m.functions[0].blocks:
        new = []
        last = None
        for inst in bb.instructions:
            tn = type(inst).__name__
            if tn == "InstLdweights":
                s_ = sig(inst)
                if last is not None and s_ == last:
                    si_ = getattr(inst, "sync_info", None)
                    if si_ is not None and (si_.on_wait or si_.on_update):
                        new.append(mybir.InstEventSemaphore(
                            name=f"ldwdedup_{nc.next_id()}",
                            engine=inst.engine, ins=[], outs=[],
                            sync_info=si_, bass_nofuse=True,
                        ))
                    removed += 1
                    continue
                last = s_
            new.append(inst)
        bb.instructions[:] = new
    return removed


def _split_multi_waits(nc):
    for bb in nc.m.functions[0].blocks:
        new = []
        for inst in bb.instructions:
            si = getattr(inst, "sync_info", None)
            if si is not None and si.on_wait is not None and len(si.on_wait) > 1:
                waits = list(si.on_wait)
                for w in waits[:-1]:
                    carrier = mybir.InstEventSemaphore(
                        name=f"waitsplit_{nc.next_id()}",
                        engine=inst.engine,
                        ins=[],
                        outs=[],
                        sync_info=mybir.SyncInfo(on_wait=[w], on_update=[]),
                        bass_nofuse=True,
                    )
                    new.append(carrier)
                si.on_wait[:] = waits[-1:]
            new.append(inst)
        bb.instructions[:] = new

N_ROWS = 8192
D = 2048
NB = 16          # row blocks
RPB = 512        # rows per block
SLOTS = 10       # blocks cached per core
N_CORES = 8
KC = D // 128    # contraction chunks
RT = RPB // 128  # 128-row subtiles per block

# Fixed slot-pair list (si = moving side in {0,1}, sj = raw/stationary
# side in 0..9). 10 slots is PROVEN minimal for a 2-mover SPMD cover:
# mixed-parity movers are forced (else odd-odd/even-even classes are
# uncovered), and the two difference-8 chains then need offsets 8 AND 9.
PAIRS = [(0, 0), (1, 1), (0, 1)]
for _k in range(2, 9):
    PAIRS += [(0, _k), (1, _k)]
PAIRS += [(1, 9)]
assert len(PAIRS) == 18


def slot_blocks(core):
    """Global block index for each slot on a given core."""
    return [(2 * core + k) % NB for k in range(SLOTS)]


def pair_weight(si, sj):
    """Host-side weight for one slot pair: diag=1, cross d<8 -> 2,
    d=8 cross pairs are computed twice globally -> 1 each."""
    if si == sj:
        return 1.0
    d = sj - si
    return 1.0 if d == 8 else 2.0


def build_program(rpb=RPB, d=D, slots=SLOTS, pairs=PAIRS, psum_bufs=5, probe=None):
    """Uniform SPMD program. Inputs (per core, host-prepped):
      xt   [slots, 128, KC, rpb] fp8 -- K-major transposed raw blocks
      iden [128, 128] bf16           -- identity mask for diag extraction
    Output: partials [128, RT*len(pairs)] f32."""
    rt = rpb // 128
    kc = d // 128
    kc8 = kc // 2
    nc = bass.Bass()
    xt_p = nc.declare_dram_parameter("xt", [slots, 128, kc, rpb], FP8,
                                     isOutput=False)
    id_p = nc.declare_dram_parameter("iden", [128, 128], BF16, isOutput=False)
    pout = nc.declare_dram_parameter(
        "partials", [128, rt * len(pairs)], F32, isOutput=True
    )

    mult = mybir.AluOpType.mult
    add = mybir.AluOpType.add
    Exp = mybir.ActivationFunctionType.Exp
    Ln = mybir.ActivationFunctionType.Ln
    DR = mybir.MatmulPerfMode.DoubleRow

    # stationary slots (sj) -> list of moving norm slots (si)
    sj_movers = {}
    for si_, sj_ in pairs:
        sj_movers.setdefault(sj_, []).append(si_)

    with TileContext(nc) as tc:
        with (
            tc.tile_pool(name="xts", bufs=1) as xt_pool,
            tc.tile_pool(name="xtns", bufs=1) as xtn_pool,
            tc.tile_pool(name="dump", bufs=3) as dump_pool,
            tc.tile_pool(name="rdump", bufs=2) as rdump_pool,
            tc.tile_pool(name="small", bufs=2) as small_pool,
            tc.tile_pool(name="acc", bufs=1) as acc_pool,
            tc.tile_pool(name="gpsum", bufs=psum_bufs, space="PSUM") as gpsum_pool,
            tc.tile_pool(name="dpsum", bufs=2, space="PSUM") as dpsum_pool,
            tc.tile_pool(name="tpsum", bufs=1, space="PSUM") as tpsum_pool,
        ):
            partials = acc_pool.tile([128, rt * len(pairs)], F32, tag="partials")
            if probe in ("noep", "noact", "mm", "mmq"):
                nc.vector.memset(partials, 0.0)
            bias_t = acc_pool.tile([128, 1], F32, tag="biasneg2")
            nc.vector.memset(bias_t, -2.0)
            # rinv16 = S*rsqrt(ssq) = exp(-0.5*ln(ssq)+ln S)   (moving scale)
            # srinv  = rsqrt(ssq)/(S/2) = exp(-0.5*ln(ssq)-ln(S/2)) (epilogue)
            lnS = acc_pool.tile([128, 1], F32, tag="lnS")
            nc.vector.memset(lnS, float(np.log(FP8_SCALE)))
            lnH = acc_pool.tile([128, 1], F32, tag="lnH")
            nc.vector.memset(lnH, float(-np.log(FP8_SCALE / 2.0)))

            iden = acc_pool.tile([128, 128], BF16, tag="iden")
            nc.gpsimd.dma_start(out=iden, in_=id_p[:, :])

            xt = [
                xt_pool.tile([128, kc, rpb], FP8, tag=f"xt{s}", name=f"xt{s}")
                for s in range(slots)
            ]
            ssq = [
                acc_pool.tile([128, rt], F32, tag=f"ssq{s}", name=f"ssq{s}")
                for s in range(slots)
            ]
            srinv = [
                acc_pool.tile([128, rt], F32, tag=f"srinv{s}",
                              name=f"srinv{s}")
                for s in range(slots)
            ]
            rinv16 = [
                acc_pool.tile([128, rt], BF16, tag=f"rinv16_{s}",
                              name=f"rinv16_{s}")
                for s in range(2)
            ]
            ones_t = acc_pool.tile([1, 128], BF16, tag="ones")
            nc.vector.memset(ones_t, 1.0)
            rrepb = [
                acc_pool.tile([128, rpb], BF16, tag=f"rrepb{s}",
                              name=f"rrepb{s}")
                for s in range(2)
            ]

            # first two slots in half-loads so their diag chains (k-outer)
            # can start on the first half; loads round-robin across the
            # three DMA-capable queues (SP, ACT HWDGE, Pool SWDGE) to
            # parallelize descriptor issue
            import os as _os
            mode = _os.environ.get("KLOADENG", "fine")
            if mode == "fine":
                # every slot split in halves, one half per queue: both
                # rings stream descriptors for the same block concurrently
                for s in range(slots):
                    nc.sync.dma_start(out=xt[s][:, : kc // 2, :],
                                      in_=xt_p[s, :, : kc // 2, :])
                    nc.gpsimd.dma_start(out=xt[s][:, kc // 2:, :],
                                        in_=xt_p[s, :, kc // 2:, :])
            else:
                engs = {
                    "sp": [nc.sync],
                    "spact": [nc.sync, nc.scalar],
                    "sppool": [nc.sync, nc.gpsimd],
                    "three": [nc.sync, nc.gpsimd, nc.scalar],
                }[mode]
                for s in range(2):
                    eng = engs[s % len(engs)]
                    eng.dma_start(out=xt[s][:, : kc // 2, :],
                                  in_=xt_p[s, :, : kc // 2, :])
                    eng.dma_start(out=xt[s][:, kc // 2:, :],
                                  in_=xt_p[s, :, kc // 2:, :])
                for s in range(2, slots):
                    eng = engs[s % len(engs)]
                    eng.dma_start(out=xt[s], in_=xt_p[s])

            # ---- raw diag Gram subtiles -> ssq, split into mm / reduce ----
            # One PSUM bank per slot holds all 4 [128,128] mi-subtiles; only
            # the very first matmul uses start=True so the bank-granular
            # PSUM zero region is armed exactly once.
            dtile = {}

            def diag_mm(s):
                # k outer so the chain can start on the first half-load
                dt_ = dpsum_pool.tile([128, rt * 128], F32, tag="dt",
                                      name=f"dt{s}")
                dtile[s] = dt_
                for k in range(kc8):
                    for mi in range(rt):
                        sl = xt[s][:, 2 * k: 2 * k + 2,
                                   mi * 128: (mi + 1) * 128]
                        nc.tensor.matmul(
                            dt_[:, mi * 128: (mi + 1) * 128], sl, sl,
                            start=(mi == 0 and k == 0),
                            stop=(mi == rt - 1 and k == kc8 - 1),
                            perf_mode=DR,
                            skip_group_check=True,
                        )

            def diag_red(s):
                # mask-by-identity with fused accumulate: one DVE
                # scalar_tensor_tensor per [128,128] subtile (this walrus
                # rejects InstTensorTensorReduce, but stt+accum compiles)
                for mi in range(rt):
                    rd = rdump_pool.tile([128, 128], BF16, tag="rdump")
                    nc.vector.scalar_tensor_tensor(
                        out=rd, in0=dtile[s][:, mi * 128: (mi + 1) * 128],
                        scalar=1.0, in1=iden, op0=mult, op1=mult,
                        accum_out=ssq[s][:, mi: mi + 1],
                    )

            def rinv_chain(s):
                lssq = small_pool.tile([128, rt], F32, tag="lssq")
                nc.scalar.activation(lssq, ssq[s], Ln)
                nc.scalar.activation(srinv[s], lssq, Exp, scale=-0.5,
                                     bias=lnH)
                if s < 2:
                    nc.scalar.activation(rinv16[s], lssq, Exp, scale=-0.5,
                                         bias=lnS)

            if probe in ("nodiag", "mm", "mmq"):
                for s_ in range(slots):
                    nc.vector.memset(ssq[s_], float(D))
                for s_ in range(slots):
                    rinv_chain(s_)
            else:
                diag_mm(0)
                diag_mm(1)
                diag_red(0)
                rinv_chain(0)
                diag_red(1)
                rinv_chain(1)
                diag_mm(2)
                diag_mm(3)

            # ---- replicated 16*rinv rows for the 2 own (moving) slots ----
            # rinv16 [:, mi] --PE transpose--> [1, 128] --K=1 ones matmul-->
            # PSUM [128, rpb] replicated --> SBUF bf16 rrepb[s]. Both matmul
            # operands stay RAW; the moving-side norm is applied per pair
            # tile in the epilogue (DVE mult by rrepb, then ACT exp).
            rT_sb = {}
            for s in range(2):
                for mi in range(rt):
                    rT = tpsum_pool.tile([1, 128], BF16, tag="rT")
                    nc.tensor.matmul(rT, rinv16[s][:, mi: mi + 1], iden,
                                     is_transpose=True, start=True, stop=True)
                    rsb = small_pool.tile([1, 128], BF16, tag="rsb",
                                          name=f"rsb{s}_{mi}")
                    nc.vector.tensor_copy(rsb, rT)
                    rT_sb[(s, mi)] = rsb
            for s in range(2):
                rp = gpsum_pool.tile([128, rpb], F32, tag="g",
                                     name=f"rrep{s}")
                for mi in range(rt):
                    nc.tensor.matmul(
                        rp[:, mi * 128: (mi + 1) * 128], ones_t,
                        rT_sb[(s, mi)],
                        start=(mi == 0), stop=(mi == rt - 1),
                        skip_group_check=True,
                    )
                nc.vector.tensor_copy(rrepb[s], rp)

            # ---- pair matmuls + epilogue, staggered with the ----
            # ---- remaining diag chains                       ----
            # Early sweeps (sj < XTN_FROM) use RAW moving tiles and a
            # per-tile DVE mult by rrepb before the exp; later sweeps use
            # pre-normalized fp8 moving tiles (xtn), whose production on
            # DVE is spread across the early iterations so it never gates
            # the PE.
            XTN_FROM = 4
            xtn = [
                xtn_pool.tile([128, kc, rpb], FP8, tag=f"xtn{s}",
                              name=f"xtn{s}")
                for s in range(2)
            ]

            def xtn_mults(kk):
                for s in range(2):
                    nc.vector.tensor_tensor(
                        out=xtn[s][:, 2 * kk: 2 * kk + 2, :],
                        in0=xt[s][:, 2 * kk: 2 * kk + 2, :],
                        in1=rrepb[s].unsqueeze(1).broadcast_to((128, 2, rpb)),
                        op=mult,
                    )

            def pair_sweep(sj):
                movers = sj_movers.get(sj, [])
                raw = sj < XTN_FROM
                for mi in range(rt):
                    gs = []
                    for si in movers:
                        gt = gpsum_pool.tile(
                            [128, rpb], F32, tag="g", name=f"g{si}_{sj}_{mi}"
                        )
                        gs.append(gt)
                    mw = 128 if probe == "mmq" else rpb
                    for k in range(kc8):
                        sl = xt[sj][:, 2 * k: 2 * k + 2,
                                    mi * 128: (mi + 1) * 128]
                        for j, si in enumerate(movers):
                            mv = xt[si] if raw else xtn[si]
                            nc.tensor.matmul(
                                gs[j][:, :mw], sl,
                                mv[:, 2 * k: 2 * k + 2, :mw],
                                start=(k == 0), stop=(k == kc8 - 1),
                                perf_mode=DR,
                            )
                    for j, si in enumerate(movers):
                        if probe in ("noep", "mm", "mmq"):
                            gd = dump_pool.tile([128, rpb], BF16, tag="dump")
                            nc.vector.tensor_copy(
                                gd[:, :128] if probe == "mmq" else gd,
                                gs[j][:, :128] if probe == "mmq" else gs[j])
                            continue
                        t = pairs.index((si, sj))
                        pcol = t * rt + mi
                        if raw:
                            g2 = dump_pool.tile([128, rpb], F32, tag="g2")
                            nc.vector.tensor_tensor(
                                out=g2, in0=gs[j], in1=rrepb[si], op=mult,
                            )
                            ein = g2
                        else:
                            ein = gs[j]
                        if probe == "noact":
                            continue
                        edump = dump_pool.tile([128, rpb], BF16, tag="dump")
                        nc.scalar.activation(
                            edump, ein, Exp, bias=bias_t,
                            scale=srinv[sj][:, mi: mi + 1],
                            accum_out=partials[:, pcol: pcol + 1],
                        )

            for sj in range(slots):
                if probe not in ("nodiag", "mm", "mmq") and sj + 2 < slots:
                    diag_red(sj + 2)
                    rinv_chain(sj + 2)
                pair_sweep(sj)
                if sj < XTN_FROM:
                    for kk in range(2 * sj, 2 * sj + 2):
                        xtn_mults(kk)
                if probe not in ("nodiag", "mm", "mmq") and sj + 4 < slots:
                    diag_mm(sj + 4)

            nc.sync.dma_start(out=pout[:, :], in_=partials)
    import os
    if not os.environ.get("KNODEDUP"):
        _dedup_ldweights(nc)
    _split_multi_waits(nc)
    return nc


_PROGRAM_CACHE = {}


def _get_program():
    if "nc" not in _PROGRAM_CACHE:
        _PROGRAM_CACHE["nc"] = build_program()
    return _PROGRAM_CACHE["nc"]


def shard_inputs(x):
    """x: [8192, 2048] f32 -> per-core input dicts (fp8, pre-transposed)."""
    f8np = mybir.dt.np(FP8)
    bfnp = mybir.dt.np(BF16)
    x8 = np.asarray(x, dtype=np.float32).astype(f8np)
    blocks = x8.reshape(NB, RPB, D)
    iden = np.eye(128, dtype=bfnp)
    in_maps = []
    for c in range(N_CORES):
        sel = blocks[slot_blocks(c)]  # [SLOTS, RPB, D]
        xt = np.ascontiguousarray(
            sel.reshape(SLOTS, RPB, KC, 128).transpose(0, 3, 2, 1)
        )  # [SLOTS, 128, KC, RPB]
        in_maps.append({"xt": xt, "iden": iden})
    return in_maps


def reduce_partials(results, rt=RT):
    """results: list of dicts with 'partials' [128, rt*18] f32 -> scalar."""
    w = np.array([pair_weight(si, sj) for (si, sj) in PAIRS], dtype=np.float64)
    total = 0.0
    for res in results:
        p = res["partials"].astype(np.float64).reshape(128, len(PAIRS), rt)
        total += (p.sum(axis=(0, 2)) * w).sum()
    return total / (N_ROWS * (N_ROWS - 1))


def kernel(class_centroid: np.ndarray) -> np.ndarray:
    x = np.asarray(class_centroid, dtype=np.float32)
    assert x.shape == (N_ROWS, D)
    nc = _get_program()
    in_maps = shard_inputs(x)
    out = run_bass_kernel_spmd(nc, in_maps, list(range(N_CORES)))
    total = reduce_partials(out.results)
    return np.float32(total)
